# revision 37
# baseline (speedup 1.0000x reference)
"""TextCNN-style conv layer (kernel sizes 3/4/5, EMB=300 -> DEPTH=256, bias,
ReLU, max-pool over time) as a Bass/Tile kernel for 8 Trainium2 NeuronCores.

Strategy: data-parallel over batch (8 samples per core), weights replicated.

Conv as dense-K matmuls over a host-materialized im2col matrix
Xrep[k, t] = x[t + k//300, k%300], shared by all three branches (branch n
reads rows [0, n*300), its weights zero-padded to the K-tile boundary).

fp8 e4m3 + DoubleRow: the PE virtualizes to 128x256, contracting 256 rows
per matmul (2 fp8 weights per cell), so each branch needs ceil(n*300/256)
K-tiles: 4/5/6 -> 15 matmuls per sample per depth-half vs 30 at K=128.
Both operands quantize to e4m3; measured end-to-end L2 error vs the fp32
reference is ~1.2e-2 (accumulation stays fp32 in PSUM).

Schedule: sample-group-of-4 OUTER, then branch, then r, then (half,
sample) inner: the 8 concurrent accumulations (2 halves x 4 samples)
exactly fill the 8 PSUM banks, each weight tile serves 4 consecutive
matmuls (LDWEIGHTS hides under the matmul stream), and each x tile is
consumed by both depth-halves AND all three branches before the next
sample-group needs fresh data -- so the steady DMA demand (~124GB/s)
sits far under the 358GB/s HBM rate and only the first pass's front
edge is tight.  Each DMA queue serializes its transfers on a ~1.3us
completion receipt, so the three early-critical transfers each lead
their own queue: x(sg0,r0) on sync, x(sg0,r1) on scalar, the branch-0
weights (both halves merged into one transfer) on gpsimd.  A stream of
dummy matmuls on a memset tile warms the PE HAM clock gate (1.2 ->
2.4GHz after ~3.4us of sustained activity) during the initial DMA ramp,
timed to end just as the first real data lands.

Epilogue: relu(max_i(y + b)) == max(0, max_i y + b): DVE reduce_max over
the window axis straight out of PSUM, broadcast bias add + clamp at 0,
output staged [d, branch, half, sample] per core and de-transposed on
host.  (tensor_tensor_reduce would fuse all three but hangs TRN2 here.)
"""

import numpy as np
import ml_dtypes

B, SEQ, EMB = 64, 394, 300
DEPTH = 256
NCORES = 8
BPC = B // NCORES  # samples per core
SEQP = 400  # x_t free-dim padded (zeros) so shifted loads stay in bounds
NS = (3, 4, 5)
NT8 = (4, 5, 6)  # ceil(n*300/256) 256-row K-tiles per branch
KTOT8 = 6  # distinct Xrep K-tiles (256 rows) per sample
KROWS = 256 * KTOT8  # 1536 (1500 real + 36 zero rows)
NWARM = 10  # PE clock-gate warmup matmuls (N=512), timed to the first x landing

TRACE = False
LAST_RESULT = None

_built = None


def _build_bass():
    import concourse.mybir as mybir
    import concourse.tile as tile
    from concourse import bacc
    from contextlib import ExitStack

    f32 = mybir.dt.float32
    f8 = mybir.dt.float8e4
    DR = mybir.MatmulPerfMode.DoubleRow

    nc = bacc.Bacc("TRN2", target_bir_lowering=False)
    # index sg*6 + r; dim2 is (sample-in-group, i) fused
    xt_d = nc.dram_tensor(
        "xt", (2 * KTOT8, 128, 8, SEQP), f8, kind="ExternalInput"
    )
    # both depth-halves of one branch in a single tensor (col = dh*nt + r)
    # so each weight load is one queue-slot on its DMA ring
    w_d = {
        br: nc.dram_tensor(
            f"wb{br}", (128, 2 * NT8[br], 2, 128), f8, kind="ExternalInput"
        )
        for br in range(3)
    }
    bp_d = nc.dram_tensor("bp", (128, 3, 2), f32, kind="ExternalInput")
    out_d = nc.dram_tensor("out_t", (128, 3, 2, BPC), f32, kind="ExternalOutput")

    with tile.TileContext(nc) as tc, ExitStack() as ctx:
        xpool = ctx.enter_context(tc.tile_pool(name="x", bufs=1))
        wpool = ctx.enter_context(tc.tile_pool(name="w", bufs=1))
        cpool = ctx.enter_context(tc.tile_pool(name="consts", bufs=1))
        spool = ctx.enter_context(tc.tile_pool(name="stage", bufs=1))
        pspool = ctx.enter_context(tc.tile_pool(name="ps", bufs=8, space="PSUM"))

        # PE clock-gate warmup operand: a memset tile, ready ~immediately
        # (memset on the otherwise-idle DVE so gpsimd's weight DMA leads its
        # queue).  Wide moving dim (rhs free 1024 -> N=512) keeps the PE
        # array at full duty so the HAM un-throttles after ~3us of warmup
        # and stays at 2.4GHz through the real stream.
        wu = cpool.tile([128, 2, 512], f8)
        nc.vector.memset(wu[:], 0)

        # Each DMA queue serializes its transfers on a ~1.3us completion
        # receipt, so the early-critical transfers must each sit at the HEAD
        # of their own queue: x(sg0,r0) leads sync, x(sg0,r1) leads scalar,
        # the branch-0 weights lead gpsimd.  Everything else follows in need
        # order, alternating rings.
        xts = {}

        def load_x(sg, r, eng):
            t = xpool.tile([128, 8, SEQP], f8, tag=f"x{sg}_{r}", name=f"x{sg}_{r}")
            eng.dma_start(t[:], xt_d[sg * KTOT8 + r])
            xts[sg, r] = t

        wts = {}

        def load_w(br, eng):
            wt = wpool.tile(
                [128, 2 * NT8[br], 2, 128], f8, tag=f"wb{br}", name=f"wb{br}"
            )
            eng.dma_start(wt[:], w_d[br][:])
            wts[br] = wt

        bt = cpool.tile([128, 3, 2], f32)

        # Queues stream at ~135-165GB/s each (the 16 SDMA engines are shared
        # across queues) and SWDGE adds ~2.6us first-byte lag vs HWDGE's
        # ~0.8us -- so the two first-matmul gates (branch-0 weights, x r0)
        # each lead one HWDGE ring, and everything else is laid out so each
        # transfer's estimated completion beats its first use.
        load_w(0, nc.sync)
        load_x(0, 0, nc.scalar)
        load_x(0, 1, nc.sync)
        load_x(0, 2, nc.scalar)
        load_w(1, nc.gpsimd)
        nc.gpsimd.dma_start(bt[:], bp_d[:])
        load_x(0, 3, nc.sync)
        load_x(0, 4, nc.scalar)
        load_w(2, nc.gpsimd)
        load_x(0, 5, nc.sync)
        load_x(1, 0, nc.scalar)
        load_x(1, 1, nc.sync)
        load_x(1, 2, nc.scalar)
        load_x(1, 3, nc.sync)
        load_x(1, 4, nc.scalar)
        load_x(1, 5, nc.sync)

        # Warm the HAM clock gate while the first DMAs land (PE would
        # otherwise sit idle and start cold at 1.2GHz).
        ps_wu = pspool.tile([128, 512], f32, tag="ps", name="ps_wu")
        for k in range(NWARM):
            nc.tensor.matmul(
                ps_wu[:, :512],
                lhsT=wu[:, :, :128],
                rhs=wu[:],
                start=True,
                stop=True,
                perf_mode=DR,
            )

        stage2 = spool.tile([128, 3, 2, BPC], f32)
        stage3 = spool.tile([128, 3, 2, BPC], f32)

        for sg in range(2):
            for br in range(3):
                nt = NT8[br]
                nw = SEQ - NS[br]  # windows the reference maxes over
                pss = {
                    (dh, j): pspool.tile(
                        [128, 512], f32, tag="ps", name=f"ps_{br}_{sg}_{dh}_{j}"
                    )
                    for dh in range(2)
                    for j in range(4)
                }
                for r in range(nt):
                    for dh in range(2):
                        for j in range(4):
                            nc.tensor.matmul(
                                pss[dh, j][:, :nw],
                                lhsT=wts[br][:, dh * nt + r, :, :],
                                rhs=xts[sg, r][:, 2 * j : 2 * j + 2, :nw],
                                start=(r == 0),
                                stop=(r == nt - 1),
                                perf_mode=DR,
                            )
                for dh in range(2):
                    for j in range(4):
                        s = sg * 4 + j
                        nc.vector.reduce_max(
                            stage2[:, br, dh, s : s + 1],
                            pss[dh, j][:, :nw],
                            axis=mybir.AxisListType.X,
                        )
            # Bias + relu for this sample-group's slice as soon as its 24
            # reduces land -- the sg0 half hides under the sg1 matmul
            # stream, leaving only a [128,3,2,4] pass on the final tail.
            sl = slice(sg * 4, sg * 4 + 4)
            nc.vector.tensor_tensor(
                stage3[:, :, :, sl],
                stage2[:, :, :, sl],
                bt[:, :, :, None].to_broadcast((128, 3, 2, 4)),
                mybir.AluOpType.add,
            )
            nc.vector.tensor_scalar_max(
                stage3[:, :, :, sl], stage3[:, :, :, sl], 0.0
            )

        nc.sync.dma_start(out_d[:], stage3[:])

    nc.compile()
    return nc


def _pack_inputs(input, W1, W2, W3, b1, b2, b3):
    f8 = ml_dtypes.float8_e4m3

    # Host-materialized im2col: Xrep[b, k, t] = x[b, t + k//300, k%300],
    # SEQ padded to 400 with zeros, K padded to 1536 with zero rows.
    xt = np.zeros((B, EMB, SEQP), np.float32)
    xt[:, :, :SEQ] = np.asarray(input, np.float32).transpose(0, 2, 1)
    xrep = np.zeros((B, KROWS, SEQP), np.float32)
    for j in range(5):
        xrep[:, j * EMB : (j + 1) * EMB, : SEQP - j] = xt[:, :, j:]
    # global row c = 256r + 128i + p  ->  [b, r, p, i, t]
    x8 = (
        xrep.reshape(B, KTOT8, 2, 128, SEQP)
        .transpose(0, 1, 3, 2, 4)
        .astype(f8)
    )  # [B, 6, 128, 2, 400]

    ws = {}
    for br, (n, W) in enumerate(zip(NS, (W1, W2, W3))):
        Wp = np.zeros((KROWS, DEPTH), np.float32)
        Wp[: n * EMB] = np.asarray(W, np.float32).T
        v = Wp.reshape(KTOT8, 2, 128, 2, 128)  # (r, i, p, dh, m)
        halves = [
            v[: NT8[br], :, :, dh, :].transpose(2, 0, 1, 3)  # (p, r, i, m)
            for dh in range(2)
        ]
        ws[br] = np.ascontiguousarray(np.concatenate(halves, axis=1)).astype(
            f8
        )  # (p, dh*nt+r, i, m)

    bp = np.empty((128, 3, 2), np.float32)
    for br, b in enumerate((b1, b2, b3)):
        b = np.asarray(b, np.float32).reshape(DEPTH)
        for dh in range(2):
            bp[:, br, dh] = b[dh * 128 : (dh + 1) * 128]
    return x8, ws, bp


def kernel(input, W1, W2, W3, b1, b2, b3):
    global _built, LAST_RESULT
    from concourse.bass_utils import run_bass_kernel_spmd

    x8, ws, bp = _pack_inputs(input, W1, W2, W3, b1, b2, b3)

    if _built is None:
        _built = _build_bass()
    nc = _built

    in_maps = []
    for c in range(NCORES):
        cx = x8[c * BPC : (c + 1) * BPC]  # [8, 6, 128, 2, 400] (s, r, p, i, t)
        # -> [sg*6+r, p, (s-in-group, i), t]
        xt = np.concatenate(
            [
                cx[sg * 4 : (sg + 1) * 4]
                .transpose(1, 2, 0, 3, 4)  # (r, p, s4, i, t)
                .reshape(KTOT8, 128, 8, SEQP)
                for sg in range(2)
            ]
        )
        m = {"xt": np.ascontiguousarray(xt), "bp": bp}
        for br in range(3):
            m[f"wb{br}"] = ws[br]
        in_maps.append(m)

    res = run_bass_kernel_spmd(
        nc, in_maps, core_ids=list(range(NCORES)), trace=TRACE
    )
    LAST_RESULT = res

    out = np.empty((B, 3 * DEPTH), np.float32)
    for c in range(NCORES):
        arr = res.results[c]["out_t"]  # [128, 3, 2, BPC]
        out[c * BPC : (c + 1) * BPC] = arr.transpose(3, 1, 2, 0).reshape(BPC, 768)
    return out


# revision 39
# speedup vs baseline: 1.0234x; 1.0234x over previous
"""TextCNN-style conv layer (kernel sizes 3/4/5, EMB=300 -> DEPTH=256, bias,
ReLU, max-pool over time) as a Bass/Tile kernel for 8 Trainium2 NeuronCores.

Strategy: data-parallel over batch (8 samples per core), weights replicated.

Conv as dense-K matmuls over a host-materialized im2col matrix
Xrep[k, t] = x[t + k//300, k%300], shared by all three branches (branch n
reads rows [0, n*300), its weights zero-padded to the K-tile boundary).

fp8 e4m3 + DoubleRow: the PE virtualizes to 128x256, contracting 256 rows
per matmul (2 fp8 weights per cell), so each branch needs ceil(n*300/256)
K-tiles: 4/5/6 -> 15 matmuls per sample per depth-half vs 30 at K=128.
Both operands quantize to e4m3; measured end-to-end L2 error vs the fp32
reference is ~1.2e-2 (accumulation stays fp32 in PSUM).

Schedule: sample-group-of-4 OUTER, then branch, then r, then (half,
sample) inner: the 8 concurrent accumulations (2 halves x 4 samples)
exactly fill the 8 PSUM banks, each weight tile serves 4 consecutive
matmuls (LDWEIGHTS hides under the matmul stream), and each x tile is
consumed by both depth-halves AND all three branches before the next
sample-group needs fresh data -- so the steady DMA demand (~124GB/s)
sits far under the 358GB/s HBM rate and only the first pass's front
edge is tight.  Each DMA queue serializes its transfers on a ~1.3us
completion receipt, so the three early-critical transfers each lead
their own queue: x(sg0,r0) on sync, x(sg0,r1) on scalar, the branch-0
weights (both halves merged into one transfer) on gpsimd.  A stream of
dummy matmuls on a memset tile warms the PE HAM clock gate (1.2 ->
2.4GHz after ~3.4us of sustained activity) during the initial DMA ramp,
timed to end just as the first real data lands.

Epilogue: relu(max_i(y + b)) == max(0, max_i y + b): DVE reduce_max over
the window axis straight out of PSUM, broadcast bias add + clamp at 0,
output staged [d, branch, half, sample] per core and de-transposed on
host.  (tensor_tensor_reduce would fuse all three but hangs TRN2 here.)
"""

import numpy as np
import ml_dtypes

B, SEQ, EMB = 64, 394, 300
DEPTH = 256
NCORES = 8
BPC = B // NCORES  # samples per core
SEQP = 400  # x_t free-dim padded (zeros) so shifted loads stay in bounds
NS = (3, 4, 5)
NT8 = (4, 5, 6)  # ceil(n*300/256) 256-row K-tiles per branch
KTOT8 = 6  # distinct Xrep K-tiles (256 rows) per sample
KROWS = 256 * KTOT8  # 1536 (1500 real + 36 zero rows)
NWARM = 9  # PE clock-gate warmup matmuls (N=512), timed to the first x landing

TRACE = False
LAST_RESULT = None

_built = None


def _build_bass():
    import concourse.mybir as mybir
    import concourse.tile as tile
    from concourse import bacc
    from contextlib import ExitStack

    f32 = mybir.dt.float32
    f8 = mybir.dt.float8e4
    DR = mybir.MatmulPerfMode.DoubleRow

    nc = bacc.Bacc("TRN2", target_bir_lowering=False)
    # index sg*6 + r; dim2 is (sample-in-group, i) fused
    xt_d = nc.dram_tensor(
        "xt", (2 * KTOT8, 128, 8, SEQP), f8, kind="ExternalInput"
    )
    # both depth-halves of one branch in a single tensor (col = dh*nt + r)
    # so each weight load is one queue-slot on its DMA ring
    w_d = {
        br: nc.dram_tensor(
            f"wb{br}", (128, 2 * NT8[br], 2, 128), f8, kind="ExternalInput"
        )
        for br in range(3)
    }
    bp_d = nc.dram_tensor("bp", (128, 3, 2), f32, kind="ExternalInput")
    out_d = nc.dram_tensor("out_t", (128, 3, 2, BPC), f32, kind="ExternalOutput")

    with tile.TileContext(nc) as tc, ExitStack() as ctx:
        xpool = ctx.enter_context(tc.tile_pool(name="x", bufs=1))
        wpool = ctx.enter_context(tc.tile_pool(name="w", bufs=1))
        cpool = ctx.enter_context(tc.tile_pool(name="consts", bufs=1))
        spool = ctx.enter_context(tc.tile_pool(name="stage", bufs=1))
        pspool = ctx.enter_context(tc.tile_pool(name="ps", bufs=8, space="PSUM"))

        # PE clock-gate warmup operand: a memset tile, ready ~immediately
        # (memset on the otherwise-idle DVE so gpsimd's weight DMA leads its
        # queue).  Wide moving dim (rhs free 1024 -> N=512) keeps the PE
        # array at full duty so the HAM un-throttles after ~3us of warmup
        # and stays at 2.4GHz through the real stream.
        wu = cpool.tile([128, 2, 512], f8)
        nc.vector.memset(wu[:], 0)

        # Each DMA queue serializes its transfers on a ~1.3us completion
        # receipt, so the early-critical transfers must each sit at the HEAD
        # of their own queue: x(sg0,r0) leads sync, x(sg0,r1) leads scalar,
        # the branch-0 weights lead gpsimd.  Everything else follows in need
        # order, alternating rings.
        xts = {}

        def load_x(sg, r, eng):
            t = xpool.tile([128, 8, SEQP], f8, tag=f"x{sg}_{r}", name=f"x{sg}_{r}")
            eng.dma_start(t[:], xt_d[sg * KTOT8 + r])
            xts[sg, r] = t

        wts = {}

        def load_w(br, eng):
            wt = wpool.tile(
                [128, 2 * NT8[br], 2, 128], f8, tag=f"wb{br}", name=f"wb{br}"
            )
            eng.dma_start(wt[:], w_d[br][:])
            wts[br] = wt

        bt = cpool.tile([128, 3, 2], f32)

        # Queues stream at ~135-165GB/s each (the 16 SDMA engines are shared
        # across queues) and SWDGE adds ~2.6us first-byte lag vs HWDGE's
        # ~0.8us -- so the two first-matmul gates (branch-0 weights, x r0)
        # each lead one HWDGE ring, and everything else is laid out so each
        # transfer's estimated completion beats its first use.
        load_w(0, nc.sync)
        load_x(0, 0, nc.scalar)
        load_x(0, 1, nc.sync)
        load_x(0, 2, nc.scalar)
        load_w(1, nc.gpsimd)
        nc.gpsimd.dma_start(bt[:], bp_d[:])
        load_x(0, 3, nc.sync)
        load_x(0, 4, nc.scalar)
        load_w(2, nc.gpsimd)
        load_x(0, 5, nc.sync)
        load_x(1, 0, nc.scalar)
        load_x(1, 1, nc.sync)
        load_x(1, 2, nc.scalar)
        load_x(1, 3, nc.sync)
        load_x(1, 4, nc.scalar)
        load_x(1, 5, nc.sync)

        # Warm the HAM clock gate while the first DMAs land (PE would
        # otherwise sit idle and start cold at 1.2GHz).
        ps_wu = pspool.tile([128, 512], f32, tag="ps", name="ps_wu")
        for k in range(NWARM):
            nc.tensor.matmul(
                ps_wu[:, :512],
                lhsT=wu[:, :, :128],
                rhs=wu[:],
                start=True,
                stop=True,
                perf_mode=DR,
            )

        stage2 = spool.tile([128, 3, 2, BPC], f32)
        stage3 = spool.tile([128, 3, 2, BPC], f32)

        for sg in range(2):
            for br in range(3):
                nt = NT8[br]
                nw = SEQ - NS[br]  # windows the reference maxes over
                pss = {
                    (dh, j): pspool.tile(
                        [128, 512], f32, tag="ps", name=f"ps_{br}_{sg}_{dh}_{j}"
                    )
                    for dh in range(2)
                    for j in range(4)
                }
                for r in range(nt):
                    for dh in range(2):
                        for j in range(4):
                            nc.tensor.matmul(
                                pss[dh, j][:, :nw],
                                lhsT=wts[br][:, dh * nt + r, :, :],
                                rhs=xts[sg, r][:, 2 * j : 2 * j + 2, :nw],
                                start=(r == 0),
                                stop=(r == nt - 1),
                                perf_mode=DR,
                            )
                for dh in range(2):
                    for j in range(4):
                        s = sg * 4 + j
                        nc.vector.reduce_max(
                            stage2[:, br, dh, s : s + 1],
                            pss[dh, j][:, :nw],
                            axis=mybir.AxisListType.X,
                        )
            # Bias + relu for this sample-group's slice as soon as its 24
            # reduces land -- the sg0 half hides under the sg1 matmul
            # stream, leaving only a [128,3,2,4] pass on the final tail.
            sl = slice(sg * 4, sg * 4 + 4)
            nc.vector.tensor_tensor(
                stage3[:, :, :, sl],
                stage2[:, :, :, sl],
                bt[:, :, :, None].to_broadcast((128, 3, 2, 4)),
                mybir.AluOpType.add,
            )
            nc.vector.tensor_scalar_max(
                stage3[:, :, :, sl], stage3[:, :, :, sl], 0.0
            )

        # Out on the idle scalar ring: the sync engine runs the end-of-NEFF
        # finalize sequence, so keeping the last DMA issue off it starts
        # the teardown ~0.65us earlier.
        nc.scalar.dma_start(out_d[:], stage3[:])

    nc.compile()
    return nc


def _pack_inputs(input, W1, W2, W3, b1, b2, b3):
    f8 = ml_dtypes.float8_e4m3

    # Host-materialized im2col: Xrep[b, k, t] = x[b, t + k//300, k%300],
    # SEQ padded to 400 with zeros, K padded to 1536 with zero rows.
    xt = np.zeros((B, EMB, SEQP), np.float32)
    xt[:, :, :SEQ] = np.asarray(input, np.float32).transpose(0, 2, 1)
    xrep = np.zeros((B, KROWS, SEQP), np.float32)
    for j in range(5):
        xrep[:, j * EMB : (j + 1) * EMB, : SEQP - j] = xt[:, :, j:]
    # global row c = 256r + 128i + p  ->  [b, r, p, i, t]
    x8 = (
        xrep.reshape(B, KTOT8, 2, 128, SEQP)
        .transpose(0, 1, 3, 2, 4)
        .astype(f8)
    )  # [B, 6, 128, 2, 400]

    ws = {}
    for br, (n, W) in enumerate(zip(NS, (W1, W2, W3))):
        Wp = np.zeros((KROWS, DEPTH), np.float32)
        Wp[: n * EMB] = np.asarray(W, np.float32).T
        v = Wp.reshape(KTOT8, 2, 128, 2, 128)  # (r, i, p, dh, m)
        halves = [
            v[: NT8[br], :, :, dh, :].transpose(2, 0, 1, 3)  # (p, r, i, m)
            for dh in range(2)
        ]
        ws[br] = np.ascontiguousarray(np.concatenate(halves, axis=1)).astype(
            f8
        )  # (p, dh*nt+r, i, m)

    bp = np.empty((128, 3, 2), np.float32)
    for br, b in enumerate((b1, b2, b3)):
        b = np.asarray(b, np.float32).reshape(DEPTH)
        for dh in range(2):
            bp[:, br, dh] = b[dh * 128 : (dh + 1) * 128]
    return x8, ws, bp


def kernel(input, W1, W2, W3, b1, b2, b3):
    global _built, LAST_RESULT
    from concourse.bass_utils import run_bass_kernel_spmd

    x8, ws, bp = _pack_inputs(input, W1, W2, W3, b1, b2, b3)

    if _built is None:
        _built = _build_bass()
    nc = _built

    in_maps = []
    for c in range(NCORES):
        cx = x8[c * BPC : (c + 1) * BPC]  # [8, 6, 128, 2, 400] (s, r, p, i, t)
        # -> [sg*6+r, p, (s-in-group, i), t]
        xt = np.concatenate(
            [
                cx[sg * 4 : (sg + 1) * 4]
                .transpose(1, 2, 0, 3, 4)  # (r, p, s4, i, t)
                .reshape(KTOT8, 128, 8, SEQP)
                for sg in range(2)
            ]
        )
        m = {"xt": np.ascontiguousarray(xt), "bp": bp}
        for br in range(3):
            m[f"wb{br}"] = ws[br]
        in_maps.append(m)

    res = run_bass_kernel_spmd(
        nc, in_maps, core_ids=list(range(NCORES)), trace=TRACE
    )
    LAST_RESULT = res

    out = np.empty((B, 3 * DEPTH), np.float32)
    for c in range(NCORES):
        arr = res.results[c]["out_t"]  # [128, 3, 2, BPC]
        out[c * BPC : (c + 1) * BPC] = arr.transpose(3, 1, 2, 0).reshape(BPC, 768)
    return out
